# revision 4
# baseline (speedup 1.0000x reference)
"""Trainium2 Bass kernel for multi-head self-attention (nn_CrossAttention).

Reference computation (B=2, S=4096, C=512, H=8 heads, Dh=64):
    q = hid @ Wq.T; k = hid @ Wk.T; v = hid @ Wv.T     (per-head split)
    out = softmax(q k^T / sqrt(Dh)) v                   (per head)
    final = concat_heads(out) @ Wo.T + bo

Sharding: batch*head parallel. 16 (batch, head) units over 8 cores ->
each core owns one batch b and two adjacent heads. Each core computes a
*partial* output projection (its two heads' contribution to final[b]);
the host sums 4 partials per batch and adds the bias.

Device-side design (engine-balance targets in parens, per forward):
  - All PE inputs are bf16 (host casts); PSUM accumulation stays f32.
    PE work: q/k/v projections ~20us, scores 109us, PV 109us, merged
    two-head output projection ~7us  -> PE ~245us, the critical path.
  - scores are computed transposed (st [kv, q]) per head into SEPARATE
    [128,512] PSUM banks, software-pipelined TWO j-chunks ahead so the
    exp latency never stalls the PE.
  - softmax: no max-pass (scores are O(1) by construction); denominator
    comes free from an appended ones-column in V; exp is split between
    the ACT engine (exact, ~5/8 of tiles) and the DVE (Schraudolph
    bit-trick int16->bf16, ~3/8 of tiles, ~+-4% on those probs).
  - normalization + output projection of q-chunk i are interleaved into
    q-chunk i+1's kv loop so they hide under the attention steady state.
  - reps>1 repeats the ENTIRE forward (loads included) for steady-state
    timing; each rep is a complete, independent forward pass.
"""

import numpy as np
import ml_dtypes

import concourse.bacc as bacc
import concourse.bass as bass
import concourse.tile as tile
from concourse import mybir
from concourse.bass_utils import run_bass_kernel_spmd

B, S, C = 2, 4096, 512
H, DH = 8, 64
HL = 2                # heads per core
DL = HL * DH          # 128, local projection width
N_CORES = 8
CC = C // 128         # 4 contraction chunks for projections
NQ = S // 512         # 8 q-chunks of 512
NJ = S // 128         # 32 kv-chunks of 128

F32 = mybir.dt.float32
BF16 = mybir.dt.bfloat16
I16 = mybir.dt.int16
EXP = mybir.ActivationFunctionType.Exp
MULT = mybir.AluOpType.mult
ADD = mybir.AluOpType.add

# Schraudolph exp(x/8) in bf16: int16(x*A8 + B7) bitcast to bf16.
# A8 folds the 1/8 softmax scale; B7 calibrated offline (max rel err ~4%).
SCH_A8 = 128.0 / (8.0 * np.log(2.0))
SCH_B7 = 16249.5


def _copy(nc, use_act, dst, src):
    # PSUM->SBUF drain on ACT (activation Copy) or DVE (tensor_copy)
    if use_act:
        nc.scalar.copy(dst, src)
    else:
        nc.vector.tensor_copy(dst, src)


def _emit_forward(tc, nc, hidT, wqT, wkT, wvT, woT, outp, use_sch=True):
    with tc.tile_pool(name="persist", bufs=1) as persist:
        qT = persist.tile([DL, S], BF16)
        kT = persist.tile([DL, S], BF16)
        v0 = persist.tile([128, NJ, DH + 1], BF16)   # V plus ones col, head 0
        v1 = persist.tile([128, NJ, DH + 1], BF16)   # head 1
        wo_sb = persist.tile([DL, C], BF16)          # both heads stacked (K=128)
        oT0 = persist.tile([DH + 1, S], F32)         # out^T accum + rowsum row
        oT1 = persist.tile([DH + 1, S], F32)
        oTn = persist.tile([DL, S], BF16)            # normalized out^T, stacked

        nc.sync.dma_start(out=wo_sb[:], in_=woT[:, :])
        # ones-columns for the rowsum rows (data cols come from projection)
        nc.gpsimd.memset(v0[:, :, DH:DH + 1], 1.0)
        nc.gpsimd.memset(v1[:, :, DH:DH + 1], 1.0)

        # ---- phase A: load hidT + weights, project q/k (transposed) and v ----
        with tc.tile_pool(name="hload", bufs=1) as hload, \
             tc.tile_pool(name="wload", bufs=1) as wload, \
             tc.tile_pool(name="pjq", bufs=2, space="PSUM") as pjq, \
             tc.tile_pool(name="pjv", bufs=4, space="PSUM") as pjv:
            hid_sb = hload.tile([128, CC, S], BF16)
            hidT_r = hidT.rearrange("(cc p) s -> p cc s", p=128)
            for cc in range(CC):
                for sh in range(2):
                    nc.sync.dma_start(
                        out=hid_sb[:, cc, sh * 2048:(sh + 1) * 2048],
                        in_=hidT_r[:, cc, sh * 2048:(sh + 1) * 2048])

            wq_sb = wload.tile([128, CC, DL], BF16)
            wk_sb = wload.tile([128, CC, DL], BF16)
            wv_sb = wload.tile([128, CC, DL], BF16)
            for w_sb, w_dram in ((wq_sb, wqT), (wk_sb, wkT), (wv_sb, wvT)):
                nc.sync.dma_start(
                    out=w_sb[:], in_=w_dram.rearrange("(cc p) d -> p cc d", p=128)
                )

            # qT/kT: psum[m,n] = sum_c W[m,c] hid[n,c] = qT[dl, s]
            for di, (dst, w_sb) in enumerate(((qT, wq_sb), (kT, wk_sb))):
                for sc in range(NQ):
                    ps = pjq.tile([DL, 512], F32, name=f"pjq{di}_{sc}", tag="pjq")
                    for cc in range(CC):
                        nc.tensor.matmul(
                            ps[:],
                            lhsT=w_sb[:, cc, :],
                            rhs=hid_sb[:, cc, sc * 512:(sc + 1) * 512],
                            start=(cc == 0),
                            stop=(cc == CC - 1),
                        )
                    _copy(nc, (di * NQ + sc) % 2 == 0,
                          dst[:, sc * 512:(sc + 1) * 512], ps[:])

            # v natural: psum[m,n] = sum_c hid[m,c] Wv[n,c] = v[s, dl]
            for jc in range(NJ):
                ps = pjv.tile([128, DL], F32, name=f"pjv{jc}", tag="pjv")
                for cc in range(CC):
                    nc.tensor.matmul(
                        ps[:],
                        lhsT=hid_sb[:, cc, jc * 128:(jc + 1) * 128],
                        rhs=wv_sb[:, cc, :],
                        start=(cc == 0),
                        stop=(cc == CC - 1),
                    )
                _copy(nc, jc % 2 == 0, v0[:, jc, 0:DH], ps[:, 0:DH])
                _copy(nc, jc % 2 == 0, v1[:, jc, 0:DH], ps[:, DH:DL])

        # ---- phases B+C: attention (q-chunk outer, kv inner), the
        # normalize + output-projection tail of q-chunk i interleaved into
        # q-chunk i+1's kv loop. PSUM budget (8 banks): 4x st [128,512]
        # (2-jc-ahead pipeline) + 2x [65,512] PV accum + 2x oproj [128,512].
        with tc.tile_pool(name="scps", bufs=2, space="PSUM") as scps, \
             tc.tile_pool(name="pvps", bufs=1, space="PSUM") as pvps, \
             tc.tile_pool(name="ptsb", bufs=3) as ptsb, \
             tc.tile_pool(name="norm", bufs=2) as norm, \
             tc.tile_pool(name="ndram", bufs=2, space="DRAM") as ndram, \
             tc.tile_pool(name="opps", bufs=2, space="PSUM") as opps, \
             tc.tile_pool(name="otsb", bufs=2) as otsb:

            # deferred C-tail tasks, popped inside the NEXT q-chunk's kv loop
            pending = []

            def pop_pending():
                if pending:
                    pending.pop(0)()

            def emit_norm_a(qc):
                # rowsum slices -> DRAM (flat) so they can be reshaped onto
                # 128 partitions for the DVE reciprocal
                qo = qc * 512
                tiles = []
                for h, oT in enumerate((oT0, oT1)):
                    srow = ndram.tile([1, 512], F32, name=f"srow{h}", tag=f"sr{h}")
                    nc.sync.dma_start(out=srow[:], in_=oT[DH:DH + 1, qo:qo + 512])
                    rs = norm.tile([128, 4], F32, name=f"rs{h}", tag=f"rs{h}")
                    nc.sync.dma_start(
                        out=rs[:], in_=srow[0, :].rearrange("(p f) -> p f", p=128))
                    tiles.append(rs)
                return tiles

            def emit_norm_b(qc, tiles):
                outs = []
                for h, rs in enumerate(tiles):
                    nc.vector.reciprocal(rs[:], rs[:])
                    rrow = ndram.tile([1, 512], F32, name=f"rrow{h}", tag=f"rr{h}")
                    nc.sync.dma_start(
                        out=rrow[0, :].rearrange("(p f) -> p f", p=128), in_=rs[:])
                    outs.append(rrow)
                return outs

            def emit_norm_c(qc, rrows):
                qo = qc * 512
                for h, (oT, rrow) in enumerate(zip((oT0, oT1), rrows)):
                    rb = norm.tile([DH, 512], F32, name=f"rb{h}", tag=f"rb{h}")
                    r0 = rrow[0, :]
                    bcast = bass.AP(tensor=r0.tensor, offset=r0.offset,
                                    ap=[[0, DH]] + list(r0.ap))
                    nc.sync.dma_start(out=rb[:], in_=bcast)
                    nc.vector.tensor_mul(oTn[h * DH:(h + 1) * DH, qo:qo + 512],
                                         oT[0:DH, qo:qo + 512], rb[:])

            def emit_oproj(sc0, n=2):
                # po[s,c] = sum_dl oTn[dl, s] wo[dl, c]  (both heads, K=128)
                for sc in range(sc0, sc0 + n):
                    po = opps.tile([128, C], F32, name=f"po{sc % 4}", tag="po")
                    nc.tensor.matmul(po[:], lhsT=oTn[:, sc * 128:(sc + 1) * 128],
                                     rhs=wo_sb[:], start=True, stop=True)
                    ot = otsb.tile([128, C], F32, name=f"ot{sc % 4}", tag="ot")
                    _copy(nc, sc % 2 == 0, ot[:], po[:])
                    nc.sync.dma_start(out=outp[sc * 128:(sc + 1) * 128, :], in_=ot[:])

            for qc in range(NQ):
                qo = qc * 512
                pva = [pvps.tile([DH + 1, 512], F32, name=f"pvacc{h}",
                                 tag=f"pv{h}") for h in range(HL)]

                def emit_scores(jc):
                    # per head: st[kv,q] in its own PSUM bank
                    sts = []
                    for h in range(HL):
                        hp = h * DH
                        st = scps.tile([128, 512], F32, name=f"st{h}_{jc % 2}",
                                       tag=f"st{h}")
                        nc.tensor.matmul(
                            st[:],
                            lhsT=kT[hp:hp + DH, jc * 128:(jc + 1) * 128],
                            rhs=qT[hp:hp + DH, qo:qo + 512],
                            start=True,
                            stop=True,
                        )
                        sts.append(st)
                    return sts

                def emit_exp(jc, sts):
                    # exp(score/8) -> bf16 probs; engine split ACT 5 : DVE 3
                    idx = qc * NJ + jc
                    dve = use_sch and ((idx * 3) % 8 >= 5)
                    pts = []
                    for h, st in enumerate(sts):
                        pt = ptsb.tile([128, 512], I16, name=f"pt{h}_{jc % 3}",
                                       tag=f"pt{h}")
                        if dve:
                            nc.vector.tensor_scalar(
                                pt[:], st[:], SCH_A8, SCH_B7, MULT, ADD)
                        else:
                            nc.scalar.activation(pt[:].bitcast(BF16), st[:],
                                                 EXP, scale=0.125)
                        pts.append(pt)
                    return pts

                def emit_pv(jc, pts):
                    for h, (vh, pt) in enumerate(zip((v0, v1), pts)):
                        nc.tensor.matmul(
                            pva[h][:],
                            lhsT=vh[:, jc, :],
                            rhs=pt[:].bitcast(BF16),
                            start=(jc == 0),
                            stop=(jc == NJ - 1),
                        )

                # software pipeline: scores 2 ahead, exp 1 ahead of PV
                stage_st = {0: emit_scores(0), 1: emit_scores(1)}
                stage_pt = {0: emit_exp(0, stage_st.pop(0))}
                for jc in range(NJ):
                    if jc + 2 < NJ:
                        stage_st[jc + 2] = emit_scores(jc + 2)
                    if jc + 1 < NJ:
                        stage_pt[jc + 1] = emit_exp(jc + 1, stage_st.pop(jc + 1))
                    emit_pv(jc, stage_pt.pop(jc))
                    if jc in (2, 8, 14, 20, 26):
                        pop_pending()
                for h, oT in enumerate((oT0, oT1)):
                    _copy(nc, qc % 2 == 0, oT[:, qo:qo + 512], pva[h][:])

                state = {}
                pending.append(lambda q=qc: state.__setitem__('a', emit_norm_a(q)))
                pending.append(lambda q=qc: state.__setitem__('b', emit_norm_b(q, state.pop('a'))))
                pending.append(lambda q=qc: emit_norm_c(q, state.pop('b')))
                pending.append(lambda s=4 * qc: emit_oproj(s, 2))
                pending.append(lambda s=4 * qc + 2: emit_oproj(s, 2))

            while pending:
                pop_pending()


def _emit(tc, nc, hidT, wqT, wkT, wvT, woT, outp, reps=1, use_sch=True):
    for _ in range(reps):
        _emit_forward(tc, nc, hidT, wqT, wkT, wvT, woT, outp, use_sch=use_sch)


def build_nc(reps=1, use_sch=True):
    nc = bacc.Bacc("TRN2", target_bir_lowering=False, debug=False)
    hidT = nc.dram_tensor("hidT", [C, S], BF16, kind="ExternalInput").ap()
    wqT = nc.dram_tensor("wqT", [C, DL], BF16, kind="ExternalInput").ap()
    wkT = nc.dram_tensor("wkT", [C, DL], BF16, kind="ExternalInput").ap()
    wvT = nc.dram_tensor("wvT", [C, DL], BF16, kind="ExternalInput").ap()
    woT = nc.dram_tensor("woT", [DL, C], BF16, kind="ExternalInput").ap()
    outp = nc.dram_tensor("outp", [S, C], F32, kind="ExternalOutput").ap()
    with tile.TileContext(nc) as tc:
        _emit(tc, nc, hidT, wqT, wkT, wvT, woT, outp, reps=reps, use_sch=use_sch)
    nc.compile()
    return nc


def make_in_maps(hidden_states, Wq, Wk, Wv, Wo):
    """Shard the full inputs into 8 per-core input maps (cast to bf16)."""

    def bf(x):
        return np.ascontiguousarray(np.asarray(x, np.float32)).astype(
            ml_dtypes.bfloat16)

    hs = np.asarray(hidden_states, dtype=np.float32)
    hidT_b = [bf(hs[b].T) for b in range(B)]
    in_maps = []
    for core in range(N_CORES):
        b = core // 4
        p = core % 4
        lo, hi = 2 * p * DH, (2 * p + 2) * DH
        in_maps.append({
            "hidT": hidT_b[b],
            "wqT": bf(np.asarray(Wq, np.float32)[lo:hi, :].T),
            "wkT": bf(np.asarray(Wk, np.float32)[lo:hi, :].T),
            "wvT": bf(np.asarray(Wv, np.float32)[lo:hi, :].T),
            "woT": bf(np.asarray(Wo, np.float32)[:, lo:hi].T),
        })
    return in_maps


def gather_output(results, bo):
    """Sum the 4 per-core partial projections per batch, add bias."""
    bo = np.asarray(bo, np.float32)
    out = np.empty((B, S, C), np.float32)
    for b in range(B):
        acc = results[4 * b]["outp"].astype(np.float32).copy()
        for p in range(1, 4):
            acc += results[4 * b + p]["outp"]
        out[b] = acc + bo
    return out


_NC_CACHE = None


def _get_nc():
    global _NC_CACHE
    if _NC_CACHE is None:
        _NC_CACHE = build_nc()
    return _NC_CACHE


def kernel(hidden_states, Wq, Wk, Wv, Wo, bo, _trace=False, _res_out=None):
    nc = _get_nc()
    in_maps = make_in_maps(hidden_states, Wq, Wk, Wv, Wo)
    res = run_bass_kernel_spmd(nc, in_maps, list(range(N_CORES)), trace=_trace)
    if _res_out is not None:
        _res_out.append(res)
    return gather_output(res.results, bo)


# revision 5
# speedup vs baseline: 1.0606x; 1.0606x over previous
"""Trainium2 Bass kernel for multi-head self-attention (nn_CrossAttention).

Reference computation (B=2, S=4096, C=512, H=8 heads, Dh=64):
    q = hid @ Wq.T; k = hid @ Wk.T; v = hid @ Wv.T     (per-head split)
    out = softmax(q k^T / sqrt(Dh)) v                   (per head)
    final = concat_heads(out) @ Wo.T + bo

Sharding: batch*head parallel. 16 (batch, head) units over 8 cores ->
each core owns one batch b and two adjacent heads. Each core computes a
*partial* output projection (its two heads' contribution to final[b]);
the host sums 4 partials per batch and adds the bias.

Device-side design (engine-balance targets in parens, per forward):
  - All PE inputs are bf16 (host casts); PSUM accumulation stays f32.
    PE work: q/k/v projections ~20us, scores 109us, PV 109us, merged
    two-head output projection ~7us  -> PE ~245us, the critical path.
  - scores are computed transposed (st [kv, q]) per head into SEPARATE
    [128,512] PSUM banks, software-pipelined TWO j-chunks ahead so the
    exp latency never stalls the PE.
  - softmax: no max-pass (scores are O(1) by construction); denominator
    comes free from an appended ones-column in V; exp is split between
    the ACT engine (exact, ~5/8 of tiles) and the DVE (Schraudolph
    bit-trick int16->bf16, ~3/8 of tiles, ~+-4% on those probs).
  - normalization + output projection of q-chunk i are interleaved into
    q-chunk i+1's kv loop so they hide under the attention steady state.
  - reps>1 repeats the ENTIRE forward (loads included) for steady-state
    timing; each rep is a complete, independent forward pass.
"""

import numpy as np
import ml_dtypes

import concourse.bacc as bacc
import concourse.bass as bass
import concourse.tile as tile
from concourse import mybir
from concourse.bass_utils import run_bass_kernel_spmd

B, S, C = 2, 4096, 512
H, DH = 8, 64
HL = 2                # heads per core
DL = HL * DH          # 128, local projection width
N_CORES = 8
CC = C // 128         # 4 contraction chunks for projections
NQ = S // 512         # 8 q-chunks of 512
NJ = S // 128         # 32 kv-chunks of 128

F32 = mybir.dt.float32
BF16 = mybir.dt.bfloat16
I16 = mybir.dt.int16
EXP = mybir.ActivationFunctionType.Exp
MULT = mybir.AluOpType.mult
ADD = mybir.AluOpType.add

# Schraudolph exp(x/8) in bf16: int16(x*A8 + B7) bitcast to bf16.
# A8 folds the 1/8 softmax scale; B7 calibrated offline (max rel err ~4%).
SCH_A8 = 128.0 / (8.0 * np.log(2.0))
SCH_B7 = 16249.5


def _copy(nc, use_act, dst, src):
    # PSUM->SBUF drain on ACT (activation Copy) or DVE (tensor_copy)
    if use_act:
        nc.scalar.copy(dst, src)
    else:
        nc.vector.tensor_copy(dst, src)


def _emit_forward(tc, nc, hidT, wqT, wkT, wvT, woT, outp, use_sch=True):
    with tc.tile_pool(name="persist", bufs=1) as persist:
        qT = persist.tile([DL, S], BF16)
        # kTp[:, h, :]: head h's K rows on its own 64 partitions, the other
        # 64 partitions zeroed -> scores matmul runs at K=128 (the PE is
        # ~1.6x faster per column at K=128 than K=64; zeros are free).
        kTp = persist.tile([128, HL, S], BF16)
        v01 = persist.tile([128, NJ, HL, DH + 1], BF16)  # V + ones col, both heads
        wo_sb = persist.tile([DL, C], BF16)          # both heads stacked (K=128)
        oT0 = persist.tile([DH + 1, S], F32)         # out^T accum + rowsum row
        oT1 = persist.tile([DH + 1, S], F32)
        oTn = persist.tile([DL, S], BF16)            # normalized out^T, stacked

        nc.sync.dma_start(out=wo_sb[:], in_=woT[:, :])
        # ones-columns for the rowsum rows (data cols come from projection)
        nc.gpsimd.memset(v01[:, :, :, DH:DH + 1], 1.0)
        # zero the off-head halves of kTp (kTp[64:,0,:] and kTp[:64,1,:])
        nc.gpsimd.memset(kTp[DH:128, 0, :], 0.0)
        nc.gpsimd.memset(kTp[0:DH, 1, :], 0.0)

        # ---- phase A: load hidT + weights, project q/k (transposed) and v ----
        with tc.tile_pool(name="hload", bufs=1) as hload, \
             tc.tile_pool(name="wload", bufs=1) as wload, \
             tc.tile_pool(name="pjq", bufs=2, space="PSUM") as pjq, \
             tc.tile_pool(name="pjv", bufs=4, space="PSUM") as pjv:
            hid_sb = hload.tile([128, CC, S], BF16)
            hidT_r = hidT.rearrange("(cc p) s -> p cc s", p=128)
            for cc in range(CC):
                for sh in range(2):
                    nc.sync.dma_start(
                        out=hid_sb[:, cc, sh * 2048:(sh + 1) * 2048],
                        in_=hidT_r[:, cc, sh * 2048:(sh + 1) * 2048])

            wq_sb = wload.tile([128, CC, DL], BF16)
            wk_sb = wload.tile([128, CC, DL], BF16)
            wv_sb = wload.tile([128, CC, DL], BF16)
            for w_sb, w_dram in ((wq_sb, wqT), (wk_sb, wkT), (wv_sb, wvT)):
                nc.sync.dma_start(
                    out=w_sb[:], in_=w_dram.rearrange("(cc p) d -> p cc d", p=128)
                )

            # qT/kT: psum[m,n] = sum_c W[m,c] hid[n,c] = qT[dl, s]
            for di, w_sb in enumerate((wq_sb, wk_sb)):
                for sc in range(NQ):
                    ps = pjq.tile([DL, 512], F32, name=f"pjq{di}_{sc}", tag="pjq")
                    for cc in range(CC):
                        nc.tensor.matmul(
                            ps[:],
                            lhsT=w_sb[:, cc, :],
                            rhs=hid_sb[:, cc, sc * 512:(sc + 1) * 512],
                            start=(cc == 0),
                            stop=(cc == CC - 1),
                        )
                    use_act = (di * NQ + sc) % 2 == 0
                    if di == 0:
                        _copy(nc, use_act, qT[:, sc * 512:(sc + 1) * 512], ps[:])
                    else:
                        _copy(nc, use_act,
                              kTp[0:DH, 0, sc * 512:(sc + 1) * 512], ps[0:DH, :])
                        _copy(nc, not use_act,
                              kTp[DH:128, 1, sc * 512:(sc + 1) * 512], ps[DH:128, :])

            # v natural: psum[m,n] = sum_c hid[m,c] Wv[n,c] = v[s, dl]
            for jc in range(NJ):
                ps = pjv.tile([128, DL], F32, name=f"pjv{jc}", tag="pjv")
                for cc in range(CC):
                    nc.tensor.matmul(
                        ps[:],
                        lhsT=hid_sb[:, cc, jc * 128:(jc + 1) * 128],
                        rhs=wv_sb[:, cc, :],
                        start=(cc == 0),
                        stop=(cc == CC - 1),
                    )
                _copy(nc, jc % 2 == 0, v01[:, jc, :, 0:DH],
                      ps[:].rearrange("p (h d) -> p h d", h=HL))

        # ---- phases B+C: attention (q-chunk outer, kv inner), the
        # normalize + output-projection tail of q-chunk i interleaved into
        # q-chunk i+1's kv loop. PSUM budget (8 banks): 4x st [128,512]
        # (2-jc-ahead pipeline) + 2x [65,512] PV accum + 2x oproj [128,512].
        with tc.tile_pool(name="scps", bufs=2, space="PSUM") as scps, \
             tc.tile_pool(name="pvps", bufs=1, space="PSUM") as pvps, \
             tc.tile_pool(name="ptsb", bufs=3) as ptsb, \
             tc.tile_pool(name="norm", bufs=2) as norm, \
             tc.tile_pool(name="ndram", bufs=2, space="DRAM") as ndram, \
             tc.tile_pool(name="opps", bufs=2, space="PSUM") as opps, \
             tc.tile_pool(name="otsb", bufs=2) as otsb:

            # deferred C-tail tasks, popped inside the NEXT q-chunk's kv loop
            pending = []

            def pop_pending():
                if pending:
                    pending.pop(0)()

            def emit_norm_a(qc):
                # rowsum slices -> DRAM (flat) so they can be reshaped onto
                # 128 partitions for the DVE reciprocal
                qo = qc * 512
                tiles = []
                for h, oT in enumerate((oT0, oT1)):
                    srow = ndram.tile([1, 512], F32, name=f"srow{h}", tag=f"sr{h}")
                    nc.sync.dma_start(out=srow[:], in_=oT[DH:DH + 1, qo:qo + 512])
                    rs = norm.tile([128, 4], F32, name=f"rs{h}", tag=f"rs{h}")
                    nc.sync.dma_start(
                        out=rs[:], in_=srow[0, :].rearrange("(p f) -> p f", p=128))
                    tiles.append(rs)
                return tiles

            def emit_norm_b(qc, tiles):
                outs = []
                for h, rs in enumerate(tiles):
                    nc.vector.reciprocal(rs[:], rs[:])
                    rrow = ndram.tile([1, 512], F32, name=f"rrow{h}", tag=f"rr{h}")
                    nc.sync.dma_start(
                        out=rrow[0, :].rearrange("(p f) -> p f", p=128), in_=rs[:])
                    outs.append(rrow)
                return outs

            def emit_norm_c(qc, rrows):
                qo = qc * 512
                for h, (oT, rrow) in enumerate(zip((oT0, oT1), rrows)):
                    rb = norm.tile([DH, 512], F32, name=f"rb{h}", tag=f"rb{h}")
                    r0 = rrow[0, :]
                    bcast = bass.AP(tensor=r0.tensor, offset=r0.offset,
                                    ap=[[0, DH]] + list(r0.ap))
                    nc.sync.dma_start(out=rb[:], in_=bcast)
                    nc.vector.tensor_mul(oTn[h * DH:(h + 1) * DH, qo:qo + 512],
                                         oT[0:DH, qo:qo + 512], rb[:])

            def emit_oproj(sc0, n=2):
                # po[s,c] = sum_dl oTn[dl, s] wo[dl, c]  (both heads, K=128)
                for sc in range(sc0, sc0 + n):
                    po = opps.tile([128, C], F32, name=f"po{sc % 4}", tag="po")
                    nc.tensor.matmul(po[:], lhsT=oTn[:, sc * 128:(sc + 1) * 128],
                                     rhs=wo_sb[:], start=True, stop=True)
                    ot = otsb.tile([128, C], F32, name=f"ot{sc % 4}", tag="ot")
                    _copy(nc, sc % 2 == 0, ot[:], po[:])
                    nc.sync.dma_start(out=outp[sc * 128:(sc + 1) * 128, :], in_=ot[:])

            for qc in range(NQ):
                qo = qc * 512
                pva = [pvps.tile([DH + 1, 512], F32, name=f"pvacc{h}",
                                 tag=f"pv{h}") for h in range(HL)]

                def emit_scores(jc):
                    # per head: st[kv,q], K=128 (off-head rows of kTp are 0)
                    sts = []
                    for h in range(HL):
                        st = scps.tile([128, 512], F32, name=f"st{h}_{jc % 2}",
                                       tag=f"st{h}")
                        nc.tensor.matmul(
                            st[:],
                            lhsT=kTp[:, h, jc * 128:(jc + 1) * 128],
                            rhs=qT[:, qo:qo + 512],
                            start=True,
                            stop=True,
                        )
                        sts.append(st)
                    return sts

                def emit_exp(jc, sts):
                    # exp(score/8) -> bf16 probs; engine split ACT 5 : DVE 3
                    idx = qc * NJ + jc
                    dve = use_sch and ((idx % 7) in (2, 4, 6))
                    pts = []
                    for h, st in enumerate(sts):
                        pt = ptsb.tile([128, 512], I16, name=f"pt{h}_{jc % 3}",
                                       tag=f"pt{h}")
                        if dve:
                            nc.vector.tensor_scalar(
                                pt[:], st[:], SCH_A8, SCH_B7, MULT, ADD)
                        else:
                            nc.scalar.activation(pt[:].bitcast(BF16), st[:],
                                                 EXP, scale=0.125)
                        pts.append(pt)
                    return pts

                def emit_pv(jc, pts):
                    for h, pt in enumerate(pts):
                        nc.tensor.matmul(
                            pva[h][:],
                            lhsT=v01[:, jc, h, :],
                            rhs=pt[:].bitcast(BF16),
                            start=(jc == 0),
                            stop=(jc == NJ - 1),
                        )

                # software pipeline: scores 2 ahead, exp 1 ahead of PV
                stage_st = {0: emit_scores(0), 1: emit_scores(1)}
                stage_pt = {0: emit_exp(0, stage_st.pop(0))}
                for jc in range(NJ):
                    if jc + 2 < NJ:
                        stage_st[jc + 2] = emit_scores(jc + 2)
                    if jc + 1 < NJ:
                        stage_pt[jc + 1] = emit_exp(jc + 1, stage_st.pop(jc + 1))
                    emit_pv(jc, stage_pt.pop(jc))
                    if jc in (2, 8, 14, 20, 26):
                        pop_pending()
                for h, oT in enumerate((oT0, oT1)):
                    _copy(nc, qc % 2 == 0, oT[:, qo:qo + 512], pva[h][:])

                state = {}
                pending.append(lambda q=qc: state.__setitem__('a', emit_norm_a(q)))
                pending.append(lambda q=qc: state.__setitem__('b', emit_norm_b(q, state.pop('a'))))
                pending.append(lambda q=qc: emit_norm_c(q, state.pop('b')))
                pending.append(lambda s=4 * qc: emit_oproj(s, 2))
                pending.append(lambda s=4 * qc + 2: emit_oproj(s, 2))

            while pending:
                pop_pending()


def _emit(tc, nc, hidT, wqT, wkT, wvT, woT, outp, reps=1, use_sch=True):
    for _ in range(reps):
        _emit_forward(tc, nc, hidT, wqT, wkT, wvT, woT, outp, use_sch=use_sch)


def build_nc(reps=1, use_sch=True):
    nc = bacc.Bacc("TRN2", target_bir_lowering=False, debug=False)
    hidT = nc.dram_tensor("hidT", [C, S], BF16, kind="ExternalInput").ap()
    wqT = nc.dram_tensor("wqT", [C, DL], BF16, kind="ExternalInput").ap()
    wkT = nc.dram_tensor("wkT", [C, DL], BF16, kind="ExternalInput").ap()
    wvT = nc.dram_tensor("wvT", [C, DL], BF16, kind="ExternalInput").ap()
    woT = nc.dram_tensor("woT", [DL, C], BF16, kind="ExternalInput").ap()
    outp = nc.dram_tensor("outp", [S, C], F32, kind="ExternalOutput").ap()
    with tile.TileContext(nc) as tc:
        _emit(tc, nc, hidT, wqT, wkT, wvT, woT, outp, reps=reps, use_sch=use_sch)
    nc.compile()
    return nc


def make_in_maps(hidden_states, Wq, Wk, Wv, Wo):
    """Shard the full inputs into 8 per-core input maps (cast to bf16)."""

    def bf(x):
        return np.ascontiguousarray(np.asarray(x, np.float32)).astype(
            ml_dtypes.bfloat16)

    hs = np.asarray(hidden_states, dtype=np.float32)
    hidT_b = [bf(hs[b].T) for b in range(B)]
    in_maps = []
    for core in range(N_CORES):
        b = core // 4
        p = core % 4
        lo, hi = 2 * p * DH, (2 * p + 2) * DH
        in_maps.append({
            "hidT": hidT_b[b],
            "wqT": bf(np.asarray(Wq, np.float32)[lo:hi, :].T),
            "wkT": bf(np.asarray(Wk, np.float32)[lo:hi, :].T),
            "wvT": bf(np.asarray(Wv, np.float32)[lo:hi, :].T),
            "woT": bf(np.asarray(Wo, np.float32)[:, lo:hi].T),
        })
    return in_maps


def gather_output(results, bo):
    """Sum the 4 per-core partial projections per batch, add bias."""
    bo = np.asarray(bo, np.float32)
    out = np.empty((B, S, C), np.float32)
    for b in range(B):
        acc = results[4 * b]["outp"].astype(np.float32).copy()
        for p in range(1, 4):
            acc += results[4 * b + p]["outp"]
        out[b] = acc + bo
    return out


_NC_CACHE = None


def _get_nc():
    global _NC_CACHE
    if _NC_CACHE is None:
        _NC_CACHE = build_nc()
    return _NC_CACHE


def kernel(hidden_states, Wq, Wk, Wv, Wo, bo, _trace=False, _res_out=None):
    nc = _get_nc()
    in_maps = make_in_maps(hidden_states, Wq, Wk, Wv, Wo)
    res = run_bass_kernel_spmd(nc, in_maps, list(range(N_CORES)), trace=_trace)
    if _res_out is not None:
        _res_out.append(res)
    return gather_output(res.results, bo)
